# revision 26
# baseline (speedup 1.0000x reference)
"""Trainium2 Bass kernel for GQA multi-head attention (B=2, S=2048, H=2048,
32 q heads / 8 kv heads / head_dim 64, RoPE, causal softmax, output proj).

Sharding over 8 NeuronCores: core c handles batch b=c//4 and kv-head pair
j=c%4 (kv heads 2j, 2j+1 -> q heads 8j..8j+7). All on-chip data is bf16
(PSUM accumulation f32); the host converts inputs and sums the 4 partial
output projections per batch.

Head pairing: plane m holds q head (8j+m) on partitions 0:64 and q head
(8j+4+m) on partitions 64:128, so par0 contracts against kv head 2j and
par1 against kv head 2j+1 -- the rope'd kT [128, S] is used directly as
the scores lhsT with no duplication. The causal mask is accumulated into
PSUM with an identity x tri matmul; softmax denominators come from ones
columns in the [v|1]/[1|v] tiles, with av1 written at PSUM partition
offset 63 so both heads' normalization is partition-aligned.
"""

import numpy as np

B, S, H = 2, 2048, 2048
NH, NKV, HD = 32, 8, 64
P = 128
ST = 512           # sequence tile (free dim of most matmuls)
NT = S // ST       # 4 sequence tiles
KC = H // P        # 16 contraction chunks for projections
NCORES = 8

_CACHE = {}
VARIANT = 'full'   # timing-bisect knob: full | actcopy | dvecopy | notail


def _build(reps=1, phases='ABC'):
    import concourse.bass as bass
    import concourse.mybir as mybir
    from concourse import bacc
    from concourse.tile import TileContext

    f32 = mybir.dt.float32
    bf16 = mybir.dt.bfloat16
    AF = mybir.ActivationFunctionType
    OP = mybir.AluOpType

    nc = bacc.Bacc("TRN2", target_bir_lowering=False, debug=False,
                   num_devices=NCORES)

    xT_d = nc.dram_tensor("xT", [H, S], bf16, kind="ExternalInput")
    wq_d = nc.dram_tensor("wq", [H, 512], bf16, kind="ExternalInput")
    wk_d = nc.dram_tensor("wk", [H, 128], bf16, kind="ExternalInput")
    wv_d = nc.dram_tensor("wv", [H, 128], bf16, kind="ExternalInput")
    wo_d = nc.dram_tensor("wo", [512, H], bf16, kind="ExternalInput")
    c2_d = nc.dram_tensor("c2", [P, S], bf16, kind="ExternalInput")
    s2_d = nc.dram_tensor("s2", [P, S], bf16, kind="ExternalInput")
    # 0/1 lower-triangular mask (key<=q), duplicated along a middle par dim
    tri_d = nc.dram_tensor("tri", [P, 2 * P], bf16, kind="ExternalInput")
    id_d = nc.dram_tensor("id128", [P, P], bf16, kind="ExternalInput")
    pm_d = nc.dram_tensor("perm", [P, P], bf16, kind="ExternalInput")
    out_d = nc.dram_tensor("out", [S, H], bf16, kind="ExternalOutput")

    with TileContext(nc) as tc:
        with tc.tile_pool(name="const", bufs=1) as constp, \
             tc.tile_pool(name="qkv", bufs=1) as qkvp, \
             tc.tile_pool(name="expp", bufs=6) as expp, \
             tc.tile_pool(name="nrm", bufs=8) as np_, \
             tc.tile_pool(name="outp", bufs=3) as outp:

            c2 = constp.tile([P, S], bf16)
            s2 = constp.tile([P, S], bf16)
            tri2 = constp.tile([P, 2, P], bf16)
            id128 = constp.tile([P, P], bf16)
            perm = constp.tile([P, P], bf16)
            ones_col = constp.tile([P, 64], bf16)
            nc.vector.memset(ones_col[:], 1.0)

            qT = qkvp.tile([P, 4, S], bf16)      # 4 head planes
            # zero-padded score lhsT: plane 0 = [kv0; 0], plane 1 = [0; kv1].
            # K=128 matmuls run ~2x faster than K=64 on HW.
            ktp = qkvp.tile([P, 2, S], bf16)
            nc.vector.memset(ktp[0:64, 1, :], 0.0)
            nc.vector.memset(ktp[64:128, 0, :], 0.0)
            vv0 = qkvp.tile([P, KC, 65], bf16)   # [v | 1]
            vv1 = qkvp.tile([P, KC, 65], bf16)   # [v | 1]
            attnT = qkvp.tile([P, 4, S], bf16)
            wo_t = qkvp.tile([P, 4, H], bf16)
            nc.vector.memset(vv0[:, :, 64:65], 1.0)
            nc.vector.memset(vv1[:, :, 64:65], 1.0)

            for rep in range(reps):
                # ============ Phase A: QKV projection + RoPE ============
                if 'A' not in phases:
                    break
                with tc.tile_pool(name="wpool", bufs=1) as wp, \
                     tc.tile_pool(name="xpool", bufs=5) as xp, \
                     tc.tile_pool(name="ropet", bufs=9) as rp, \
                     tc.tile_pool(name="psA", bufs=5, space="PSUM") as psA, \
                     tc.tile_pool(name="psV", bufs=1, space="PSUM") as psV, \
                     tc.tile_pool(name="psT", bufs=1, space="PSUM") as psT, \
                     tc.tile_pool(name="psW", bufs=1, space="PSUM") as psW:

                    wq = wp.tile([P, KC, 512], bf16)
                    wk = wp.tile([P, KC, 128], bf16)
                    wv = wp.tile([P, KC, 128], bf16)
                    wq_view = wq_d.ap().rearrange("(ko p) m -> p ko m", p=P)
                    wk_view = wk_d.ap().rearrange("(ko p) m -> p ko m", p=P)
                    wv_view = wv_d.ap().rearrange("(ko p) m -> p ko m", p=P)
                    # critical prefix: chunk-0 weights + chunk-0 x first.
                    # sync feeds the T0 stream; scalar carries the weight
                    # remainder in parallel; wo is chunked and deferred to T1
                    # on gpsimd so its transfers can't block the criticals.
                    nc.sync.dma_start(wq[:, 0:1], wq_view[:, 0:1])
                    nc.scalar.dma_start(wk[:, 0:4], wk_view[:, 0:4])
                    nc.scalar.dma_start(wv[:, 0:4], wv_view[:, 0:4])
                    nc.scalar.dma_start(wq[:, 4:7], wq_view[:, 4:7])
                    nc.scalar.dma_start(wq[:, 7:10], wq_view[:, 7:10])
                    nc.scalar.dma_start(wk[:, 4:16], wk_view[:, 4:16])
                    nc.scalar.dma_start(wv[:, 4:16], wv_view[:, 4:16])

                    for T in range(NT):
                        ts = slice(ST * T, ST * (T + 1))
                        psq = [psA.tile([P, ST], f32, tag="proj", name=f"q{T}_{m}")
                               for m in range(4)]
                        psk = psA.tile([P, ST], f32, tag="proj", name=f"k{T}")
                        # V projection lands as [vdim, seq] (N=512 matmuls);
                        # PE-transposed back to [seq, vdim] at tile end.
                        psv = psV.tile([P, ST], f32, tag="v")
                        xg = [xp.tile([P, 4, ST], bf16, tag="xk", name=f"x{T}_{g}")
                              for g in range(4)]
                        for g in range(4):
                            src_v = (xT_d.ap()[512 * g:512 * (g + 1), ts]
                                     .rearrange("(kc p) s -> p kc s", p=P))
                            if T == 0 and g == 0:
                                nc.sync.dma_start(xg[g][:, 0:1], src_v[:, 0:1])
                                nc.sync.dma_start(wq[:, 1:4], wq_view[:, 1:4])
                                nc.sync.dma_start(xg[g][:, 1:4], src_v[:, 1:4])
                            else:
                                q = nc.sync if g < 2 else nc.scalar
                                q.dma_start(xg[g][:], src_v)
                        if T == 0:
                            # mid-priority remainder, behind T0's x on scalar
                            nc.scalar.dma_start(wq[:, 10:13], wq_view[:, 10:13])
                            nc.scalar.dma_start(wq[:, 13:16], wq_view[:, 13:16])
                            if rep == 0:
                                nc.scalar.dma_start(perm[:], pm_d.ap())
                                nc.scalar.dma_start(id128[:], id_d.ap())
                                nc.scalar.dma_start(c2[:], c2_d.ap())
                                nc.scalar.dma_start(s2[:], s2_d.ap())
                                nc.scalar.dma_start(
                                    tri2[:].rearrange("p a b -> p (a b)"),
                                    tri_d.ap())
                        if T == 2:
                            wo_view = wo_d.ap().rearrange("(cp p) e -> p cp e", p=P)
                            for wi in range(4):
                                nc.scalar.dma_start(
                                    wo_t[:, :, 512 * wi:512 * (wi + 1)],
                                    wo_view[:, :, 512 * wi:512 * (wi + 1)])
                        for k in range(KC):
                            xkc = xg[k // 4][:, k % 4]
                            st, sp = (k == 0), (k == KC - 1)
                            for m in range(4):
                                nc.tensor.matmul(psq[m][:], wq[:, k, 128 * m:128 * (m + 1)],
                                                 xkc, start=st, stop=sp)
                            nc.tensor.matmul(psk[:], wk[:, k], xkc, start=st, stop=sp)
                            nc.tensor.matmul(psv[:], wv[:, k], xkc,
                                             start=st, stop=sp)

                        # rope epilogue: frees the proj psum bank via the raw
                        # copy, rest works from SBUF/psW
                        def rope(psrc, dst, idx, dst_hi=None):
                            raw = rp.tile([P, ST], bf16, tag="raw")
                            # spread raw copies across Act/DVE/Pool so the
                            # proj psum banks free fast at tile boundaries
                            if idx < 2:
                                nc.scalar.copy(raw[:], psrc[:])
                            else:
                                nc.vector.tensor_copy(raw[:], psrc[:])
                            swp = psW.tile([P, ST], f32, tag="swp")
                            nc.tensor.matmul(swp[:], perm[:], raw[:])
                            t1 = rp.tile([P, ST], bf16, tag="t1")
                            nc.vector.tensor_tensor(t1[:], raw[:], c2[:, ts], OP.mult)
                            t2 = rp.tile([P, ST], bf16, tag="t2")
                            nc.vector.tensor_tensor(t2[:], swp[:], s2[:, ts], OP.mult)
                            if dst_hi is None:
                                nc.vector.tensor_tensor(dst, t1[:], t2[:], OP.add)
                            else:
                                nc.vector.tensor_tensor(dst, t1[0:64, :],
                                                        t2[0:64, :], OP.add)
                                nc.vector.tensor_tensor(dst_hi, t1[64:128, :],
                                                        t2[64:128, :], OP.add)

                        rope(psk, ktp[0:64, 0, ts], 0, dst_hi=ktp[64:128, 1, ts])
                        for m in range(4):
                            rope(psq[m], qT[:, m, ts], 1 + m)
                        # V: psum [vd, seq] -> sbuf -> PE transpose 128-blocks
                        # -> [seq, vd] slices of vv0/vv1
                        vts = rp.tile([P, ST], bf16, tag="raw",
                                      name=f"vts{T}")
                        nc.vector.tensor_copy(vts[:], psv[:])
                        vtp = psT.tile([P, 4, P], bf16, tag="vt")
                        for b in range(4):
                            nc.tensor.transpose(vtp[:, b, :],
                                                vts[:, 128 * b:128 * (b + 1)],
                                                id128[:])
                        for b in range(4):
                            nc.vector.tensor_copy(vv0[:, 4 * T + b, 0:64],
                                                  vtp[:, b, 0:64])
                            nc.vector.tensor_copy(vv1[:, 4 * T + b, 0:64],
                                                  vtp[:, b, 64:128])

                # ===== Phase B+C fused: attention (t-major) + output proj =====
                if 'B' not in phases:
                    continue
                with tc.tile_pool(name="psS", bufs=2, space="PSUM") as psS, \
                     tc.tile_pool(name="psV2", bufs=2, space="PSUM") as psV2:

                    def tail_pre(avt, out1, t, pair, duo):
                        """Issued right at duo end: reciprocal + av copies
                        run on DVE during the duo transition."""
                        if VARIANT == 'notail':
                            return None
                        ts = slice(ST * t, ST * (t + 1))
                        rcp = np_.tile([1, 2, ST], bf16, tag="rcp")
                        with nc.allow_low_precision(reason="softmax denom bf16"):
                            nc.vector.reciprocal(rcp[:], avt[64:65, :, :])
                        nc.vector.tensor_copy(attnT[0:64, pair, ts],
                                              avt[0:64, 0, :])
                        nc.vector.tensor_copy(out1[0:64, :], avt[0:64, 1, :])
                        return rcp

                    def tail_post(t, pair, avt, out1, rcp):
                        """At the next duo's first slot: broadcast 1/den into
                        a psS-pool tile and scale in place. avt itself was
                        fully consumed at duo end, so the next duo never
                        waits."""
                        if VARIANT == 'notail':
                            return
                        ts = slice(ST * t, ST * (t + 1))
                        psbt = psS.tile([P, 2, ST], f32, tag="sc",
                                        name=f"psb{t}_{pair}")
                        nc.tensor.matmul(psbt[0:64, 0, :], ones_col[0:1, :],
                                         rcp[:, 0])
                        nc.tensor.matmul(psbt[0:64, 1, :], ones_col[0:1, :],
                                         rcp[:, 1])
                        nc.vector.tensor_tensor(attnT[0:64, pair, ts],
                                                attnT[0:64, pair, ts],
                                                psbt[0:64, 0, :], OP.mult)
                        nc.vector.tensor_tensor(out1[0:64, :],
                                                out1[0:64, :], psbt[0:64, 1, :],
                                                OP.mult)

                    def cproj_groups(t):
                        """Yield C(t) as 8 closures, each one pso group
                        (one sb's pair of e-chunks); emitted interleaved into
                        the next t's chunk stream so Act never starves."""
                        if 'C' not in phases:
                            return
                        ots = {}

                        def group(sb, epair):
                            def emit():
                                if sb not in ots:
                                    ots[sb] = outp.tile([P, 4, ST], bf16, tag="ot",
                                                        name=f"ot{sb}")
                                ot = ots[sb]
                                pso = psS.tile([P, 2, ST], f32, tag="sc",
                                               name=f"o{sb}_{epair}")
                                for sub in range(2):
                                    e = 2 * epair + sub
                                    es = slice(ST * e, ST * (e + 1))
                                    for cp in range(4):
                                        nc.tensor.matmul(pso[:, sub],
                                                         attnT[:, cp, P * sb:P * (sb + 1)],
                                                         wo_t[:, cp, es],
                                                         start=(cp == 0), stop=(cp == 3))
                                if epair == 0:
                                    nc.vector.tensor_copy(
                                        ot[:, 0:2], pso[:])
                                else:
                                    nc.scalar.copy(ot[:, 2:4], pso[:])
                                if epair == 1:
                                    nc.gpsimd.dma_start(
                                        out_d.ap()[P * sb:P * (sb + 1), :],
                                        ot[:].rearrange("p e s -> p (e s)"))
                            return emit

                        for sb in range(4 * t, 4 * (t + 1)):
                            for epair in range(2):
                                yield group(sb, epair)

                    def scores_exp(t, pair, k, diag, c0):
                        kk = slice(P * k, P * (k + 1))
                        pss = psS.tile([P, 2, ST], f32, tag="sc")
                        for par in range(2):
                            nc.tensor.matmul(
                                pss[:, par, c0:],
                                ktp[:, par, kk],
                                qT[:, pair, ST * t + c0:ST * (t + 1)])
                        ex = expp.tile([P, 2, ST], bf16, tag="expS")
                        if VARIANT == 'actcopy':
                            nc.scalar.copy(ex[:, :, c0:], pss[:, :, c0:])
                        elif VARIANT == 'dvecopy':
                            nc.vector.tensor_copy(ex[:, :, c0:], pss[:, :, c0:])
                        else:
                            nc.scalar.activation(ex[:, :, c0:], pss[:, :, c0:],
                                                 AF.Exp, scale=0.125)
                        if diag:
                            # zero the masked (key > q) upper triangle of the
                            # 128x128 diagonal block, post-exp
                            nc.vector.tensor_tensor(
                                ex[:, :, c0:c0 + 128], ex[:, :, c0:c0 + 128],
                                tri2[:], OP.mult)
                        return ex

                    def flush_tails(tps):
                        tt = tps[0][0]
                        p0 = tps[0][1]
                        for tp in tps:
                            tail_post(*tp[:5])
                        if VARIANT != 'notail':
                            tts = slice(ST * tt, ST * (tt + 1))
                            nc.sync.dma_start(
                                attnT[64:128, p0:p0 + len(tps), tts],
                                tps[0][5][0:64, 0:len(tps), :])

                    cpend = []       # C groups ready to interleave
                    cgate = []       # C groups gated until prior t's tails done
                    tails_pending = []
                    for t in range(NT):
                        ts = slice(ST * t, ST * (t + 1))
                        nch = 4 * (t + 1)
                        # two pairs' chunk streams interleaved: exp(k) gets a
                        # full extra PE slot before its AV is needed
                        for duo in range(2):
                            pairs = (2 * duo, 2 * duo + 1)
                            avts = {p: psV2.tile([P, 2, ST], f32, tag="av",
                                                 name=f"av{t}_{p}")
                                    for p in pairs}
                            pend = []   # (pair, k, c0, ex) awaiting AV
                            cclock = 0
                            for k in range(nch):
                                diag = k >= 4 * t
                                c0 = 128 * (k - 4 * t) if diag else 0
                                stash = [(p, k, c0,
                                          scores_exp(t, p, k, diag, c0))
                                         for p in pairs]
                                if k == 0 and tails_pending:
                                    # prior duo's tails: their copies ran
                                    # during the transition, psb won't stall PE
                                    flush_tails(tails_pending)
                                    if any(tp[1] == 3 for tp in tails_pending):
                                        cpend.extend(cgate)
                                        cgate = []
                                    tails_pending = []
                                for (p, pk, pc0, pex) in pend:
                                    avt = avts[p]
                                    nc.tensor.matmul(avt[0:65, 0, pc0:],
                                                     vv0[:, pk, :],
                                                     pex[:, 0, pc0:],
                                                     start=(pk == 0), stop=False)
                                    nc.tensor.matmul(avt[0:65, 1, pc0:],
                                                     vv1[:, pk, :],
                                                     pex[:, 1, pc0:],
                                                     start=(pk == 0), stop=False)
                                pend = stash
                                cclock += 1
                                if cpend and cclock % 3 == 2:
                                    cpend.pop(0)()
                            for (p, pk, pc0, pex) in pend:
                                avt = avts[p]
                                nc.tensor.matmul(avt[0:65, 0, pc0:], vv0[:, pk, :],
                                                 pex[:, 0, pc0:],
                                                 start=(pk == 0), stop=True)
                                nc.tensor.matmul(avt[0:65, 1, pc0:], vv1[:, pk, :],
                                                 pex[:, 1, pc0:],
                                                 start=(pk == 0), stop=True)
                            ot1 = np_.tile([P, 2, ST], bf16, tag="otmp",
                                           name=f"ot1_{t}_{duo}")
                            tails_pending = [
                                (t, p, avts[p], ot1[:, i],
                                 tail_pre(avts[p], ot1[:, i], t, p, duo), ot1)
                                for i, p in enumerate(pairs)]
                            if cpend:
                                cpend.pop(0)()
                        cgate = list(cproj_groups(t))
                    if tails_pending:
                        flush_tails(tails_pending)
                    cpend.extend(cgate)
                    for g in cpend:
                        g()

    nc.compile()
    return nc


def _host_prep(x, rotary_cos, rotary_sin, Wq, Wk, Wv, Wo):
    import ml_dtypes
    bf16 = ml_dtypes.bfloat16
    x = np.asarray(x, np.float32)
    cos = np.asarray(rotary_cos, np.float32)
    sin = np.asarray(rotary_sin, np.float32)
    Wq = np.asarray(Wq, np.float32)
    Wk = np.asarray(Wk, np.float32)
    Wv = np.asarray(Wv, np.float32)
    Wo = np.asarray(Wo, np.float32)

    c2 = np.empty((P, S), np.float32)
    s2 = np.empty((P, S), np.float32)
    for p in range(P):
        c2[p] = cos[:, p % 32]
        s2[p] = sin[:, p % 32] * (-1.0 if (p % 64) < 32 else 1.0)
    tri01 = (np.arange(P)[:, None] <= np.arange(P)[None, :]).astype(np.float32)
    tri = np.concatenate([tri01, tri01], axis=1)  # [P, 2P], dup for par dim
    id128 = np.eye(P, dtype=np.float32)
    # swap permutation: within each 64-dim head, swap 32-halves
    # out_row r = in_row swap(r); perm[swap(m), m] = 1 (perm is the lhsT)
    swap = np.arange(P)
    swap = (swap // 64) * 64 + ((swap + 32) % 64)
    perm = np.zeros((P, P), np.float32)
    perm[swap, np.arange(P)] = 1.0

    xTs = [np.ascontiguousarray(x[b].T).astype(bf16) for b in range(B)]
    c2 = c2.astype(bf16)
    s2 = s2.astype(bf16)
    tri = tri.astype(bf16)
    id128 = id128.astype(bf16)
    perm = perm.astype(bf16)

    in_maps = []
    for c in range(NCORES):
        b, j = divmod(c, 4)
        # plane m: q head 8j+m on dims 0:64, q head 8j+4+m on 64:128
        qcols = np.concatenate(
            [np.arange(64 * (8 * j + m), 64 * (8 * j + m) + 64).tolist()
             + np.arange(64 * (8 * j + 4 + m), 64 * (8 * j + 4 + m) + 64).tolist()
             for m in range(4)]).astype(np.int64)
        kvcols = np.arange(64 * 2 * j, 64 * (2 * j + 2))
        in_maps.append({
            "xT": xTs[b],
            "wq": np.ascontiguousarray(Wq[:, qcols]).astype(bf16),
            "wk": np.ascontiguousarray(Wk[:, kvcols]).astype(bf16),
            "wv": np.ascontiguousarray(Wv[:, kvcols]).astype(bf16),
            "wo": np.ascontiguousarray(Wo[qcols, :]).astype(bf16),
            "c2": c2, "s2": s2, "tri": tri, "id128": id128, "perm": perm,
        })
    return in_maps


def kernel(x, rotary_cos, rotary_sin, Wq, Wk, Wv, Wo, reps=1, phases='ABC', _want_res=False):
    from concourse.bass_utils import run_bass_kernel_spmd
    key = (reps, phases)
    if key not in _CACHE:
        _CACHE[key] = _build(reps, phases)
    nc = _CACHE[key]
    in_maps = _host_prep(x, rotary_cos, rotary_sin, Wq, Wk, Wv, Wo)
    res = run_bass_kernel_spmd(nc, in_maps, list(range(NCORES)))
    out = np.empty((B, S, H), np.float32)
    for b in range(B):
        acc = res.results[4 * b]["out"].astype(np.float64)
        for j in range(1, 4):
            acc += res.results[4 * b + j]["out"].astype(np.float64)
        out[b] = acc.astype(np.float32)
    if _want_res:
        return out, res
    return out

